# revision 12
# baseline (speedup 1.0000x reference)
"""BitLinear (BitNet b1.58-style) forward on 8 Trainium2 NeuronCores.

Column-parallel sharding: weight rows (out_features) split across 8
cores, activations replicated, outputs concatenated on the host.

Math (exact-integer formulation):
  X_int = clip(round(X * 128/max(rowmax|X|, EPS)), -128, 127)  [per-token]
  W_int = clip(round(W / w_scale), -1, 1)                       [ternary]
  Y     = (X_int @ W_int^T) * (rowmax_c/128) * w_scale

All input-side quantization/layout is host-side numpy prep (it is
0.1% of the FLOPs); the device program is a pure pipelined matmul.
Host pre-packs both operands in the exact SBUF layout so every load
is one large fully-contiguous DMA:

  - x  [T/TB*128, KT*TB] bf16 : row tb*128+p, col kk*TB+t holds
        X_int[tb*TB+t, kk*128+p]; one 4MB DMA per 512-token block.
  - w  [128, NC*KT*NCH] fp8e4 : col j*KT*NCH + kk*NCH + o holds
        W_int[j*NCH+o, kk*128+p]; ternary, exact in fp8.
  - osc [128, T/128] f32      : per-token output scale rowmax_c*ws/128.

Per core: 8192 bf16(stationary x-tile) x fp8(moving 512-col weight
chunk) matmuls, fp32 PSUM accumulation over K (exact: |sum| < 2^19
<< 2^24), ACT applies the per-token scale on drain, batched 1MB
y-store per 128-token tile.
"""

import sys

try:
    import concourse.bass as bass  # noqa: F401
except ImportError:
    sys.path.insert(0, "/opt/trn_rl_repo")

import numpy as np
import ml_dtypes

import concourse.tile as tile
from concourse import bacc, mybir
from concourse.bass_utils import run_bass_kernel_spmd

F32 = mybir.dt.float32
BF16 = mybir.dt.bfloat16
FP8 = mybir.dt.float8e4

EPS = 1e-5
P = 128

N_CORES = 8
B, S, K_IN, N_OUT = 4, 2048, 4096, 16384
M_FULL = B * S
N_SHARD = N_OUT // N_CORES

KT = K_IN // P  # 32 k-tiles
NCH = 512       # psum chunk (free dim per matmul)
TB = 512        # tokens per resident x slab


def build_xloop(tc, pools, wt, osc, xt_ap, y_ap, order="kk", do_xdma=True,
                do_ydma=True, x_engine="sync", y_engine="sync"):
    nc = tc.nc
    xbp, yp, psy = pools
    xeng = getattr(nc, x_engine)
    yeng = getattr(nc, y_engine)
    T = xt_ap.shape[0] // P * TB
    NB = T // TB
    O = y_ap.shape[1]
    NC = O // NCH

    xb0 = None
    if not do_xdma:
        xb0 = xbp.tile([P, KT * TB], BF16, tag="xb")
        xeng.dma_start(xb0[:], xt_ap[0:P, :])

    for tb in range(NB):
        if do_xdma:
            xb = xbp.tile([P, KT * TB], BF16, tag="xb")
            xeng.dma_start(xb[:], xt_ap[tb * P : (tb + 1) * P, :])
        else:
            xb = xb0
        for m4 in range(TB // P):
            tt = tb * (TB // P) + m4
            ysb = yp.tile([P, O], F32, tag="y")
            if order == "kk":
                for j in range(NC):
                    ps = psy.tile([P, NCH], F32)
                    for kk in range(KT):
                        nc.tensor.matmul(
                            ps[:],
                            xb[:, kk * TB + m4 * P : kk * TB + (m4 + 1) * P],
                            wt[:, (j * KT + kk) * NCH : (j * KT + kk + 1) * NCH],
                            start=(kk == 0),
                            stop=(kk == KT - 1),
                        )
                    nc.scalar.mul(
                        ysb[:, j * NCH : (j + 1) * NCH], ps[:], osc[:, tt : tt + 1]
                    )
            else:
                pss = [psy.tile([P, NCH], F32, name=f"ps{j}", tag=f"ps{j}")
                       for j in range(NC)]
                for kk in range(KT):
                    for j in range(NC):
                        nc.tensor.matmul(
                            pss[j][:],
                            xb[:, kk * TB + m4 * P : kk * TB + (m4 + 1) * P],
                            wt[:, (j * KT + kk) * NCH : (j * KT + kk + 1) * NCH],
                            start=(kk == 0),
                            stop=(kk == KT - 1),
                        )
                for j in range(NC):
                    nc.scalar.mul(
                        ysb[:, j * NCH : (j + 1) * NCH], pss[j][:],
                        osc[:, tt : tt + 1],
                    )
            if do_ydma:
                yeng.dma_start(y_ap[tt * P : (tt + 1) * P, :], ysb[:])


def build_program(M, K, N, n_cores=N_CORES, repeat=1, debug=False, xpose="dma",
                  opts=None):
    # xpose selects the matmul issue order for A/B testing:
    #   "dma" -> chunk-outer ("kk"), "pe" -> kk-outer stationary-reuse ("jr")
    opts = dict(opts or {})
    order = opts.get("order", "jr" if xpose == "pe" else "kk")
    do_xdma = opts.get("do_xdma", True)
    do_ydma = opts.get("do_ydma", True)
    x_engine = opts.get("x_engine", "sync")
    y_engine = opts.get("y_engine", "sync")
    psum_bufs = opts.get("psum_bufs", 2 if order == "jr" else 8)
    xb_bufs = opts.get("xb_bufs", 3)
    NB = M // TB
    NC = N // NCH
    nc = bacc.Bacc(
        "TRN2", target_bir_lowering=False, debug=debug, num_devices=n_cores
    )
    xt_ap = nc.dram_tensor("xt", [NB * P, KT * TB], BF16, kind="ExternalInput").ap()
    w_ap = nc.dram_tensor("w", [P, NC * KT * NCH], FP8, kind="ExternalInput").ap()
    osc_ap = nc.dram_tensor("osc", [P, M // P], F32, kind="ExternalInput").ap()
    y_ap = nc.dram_tensor("y", [M, N], F32, kind="ExternalOutput").ap()

    import contextlib

    with tile.TileContext(nc) as tc:
        ctx = contextlib.ExitStack()
        with ctx:
            const = ctx.enter_context(tc.tile_pool(name="const", bufs=1))
            wtp = ctx.enter_context(tc.tile_pool(name="wt", bufs=1))
            xbp = ctx.enter_context(tc.tile_pool(name="xb", bufs=xb_bufs))
            yp = ctx.enter_context(tc.tile_pool(name="y", bufs=3))
            psy = ctx.enter_context(
                tc.tile_pool(name="psy", bufs=psum_bufs, space="PSUM")
            )

            osc = const.tile([P, M // P], F32)
            nc.sync.dma_start(osc[:], osc_ap)
            wt = wtp.tile([P, NC * KT * NCH], FP8)
            for j in range(NC):
                nc.sync.dma_start(
                    wt[:, j * KT * NCH : (j + 1) * KT * NCH],
                    w_ap[:, j * KT * NCH : (j + 1) * KT * NCH],
                )

            pools = (xbp, yp, psy)
            kw = dict(order=order, do_xdma=do_xdma, do_ydma=do_ydma,
                      x_engine=x_engine, y_engine=y_engine)
            if repeat == 1:
                build_xloop(tc, pools, wt, osc, xt_ap, y_ap, **kw)
            else:
                with tc.For_i(0, repeat, 1) as _i:
                    build_xloop(tc, pools, wt, osc, xt_ap, y_ap, **kw)
    nc.compile()
    return nc


def make_inputs_np(x_full, weight_full, n_cores=N_CORES):
    """Host-side prep: quantize + pack inputs in device SBUF layout."""
    k_in = x_full.shape[-1]
    xm = np.ascontiguousarray(x_full.reshape(-1, k_in), dtype=np.float32)
    T = xm.shape[0]
    rm = np.max(np.abs(xm), axis=1, keepdims=True)
    rmc = np.maximum(rm, np.float32(EPS))
    s = (np.float32(128.0) / rmc).astype(np.float32)
    xq = np.clip(np.rint(xm * s), -128.0, 127.0)
    # pack: [tb, t, kk, p] -> [tb, p, kk, t]
    x4 = xq.reshape(T // TB, TB, KT, P)
    xpk = np.ascontiguousarray(np.transpose(x4, (0, 3, 2, 1))).reshape(
        T // TB * P, KT * TB
    ).astype(ml_dtypes.bfloat16)

    ws = np.float32(
        max(np.mean(np.abs(weight_full), dtype=np.float32), np.float32(EPS))
    )
    osc = (ws / s[:, 0]).astype(np.float32)  # [T]
    osc_mat = np.ascontiguousarray(osc.reshape(-1, P).T)  # [P, MT]

    wq = np.clip(np.rint(weight_full.astype(np.float32) / ws), -1.0, 1.0)
    nshard = weight_full.shape[0] // n_cores
    NC = nshard // NCH
    in_maps = []
    for c in range(n_cores):
        wc = wq[c * nshard : (c + 1) * nshard]  # [O, K]
        w4 = wc.reshape(NC, NCH, KT, P)  # [j, o, kk, p]
        wpk = np.ascontiguousarray(np.transpose(w4, (3, 0, 2, 1))).reshape(
            P, NC * KT * NCH
        ).astype(ml_dtypes.float8_e4m3)
        in_maps.append({"xt": xpk, "w": wpk, "osc": osc_mat})
    return in_maps, float(ws)


_NC_CACHE = {}
DEFAULT_XPOSE = "dma"


def _get_program():
    key = (M_FULL, K_IN, N_SHARD, N_CORES, DEFAULT_XPOSE)
    if key not in _NC_CACHE:
        _NC_CACHE[key] = build_program(
            M_FULL, K_IN, N_SHARD, N_CORES, xpose=DEFAULT_XPOSE
        )
    return _NC_CACHE[key]


def kernel(x, weight):
    x = np.asarray(x)
    weight = np.asarray(weight)
    assert x.shape == (B, S, K_IN) and weight.shape == (N_OUT, K_IN)
    nc = _get_program()
    in_maps, _ = make_inputs_np(x, weight, N_CORES)
    res = run_bass_kernel_spmd(nc, in_maps, list(range(N_CORES)))
    y = np.concatenate(
        [res.results[c]["y"] for c in range(N_CORES)], axis=1
    )
    return np.ascontiguousarray(y.reshape(B, S, N_OUT), dtype=np.float32)


# revision 14
# speedup vs baseline: 1.3688x; 1.3688x over previous
"""BitLinear (BitNet b1.58-style) forward on 8 Trainium2 NeuronCores.

Column-parallel sharding: weight rows (out_features) split across 8
cores, activations replicated, outputs concatenated on the host.

Math (exact-integer formulation):
  X_int = clip(round(X * 128/max(rowmax|X|, EPS)), -128, 127)  [per-token]
  W_int = clip(round(W / w_scale), -1, 1)                       [ternary]
  Y     = (X_int @ W_int^T) * (rowmax_c/128) * w_scale

All input-side quantization/layout is host-side numpy prep (it is
0.1% of the FLOPs); the device program is a pure pipelined matmul.
Host pre-packs both operands in the exact SBUF layout so every load
is one large fully-contiguous DMA:

  - x  [T/TB*128, KT*TB] bf16 : row tb*128+p, col kk*TB+t holds
        X_int[tb*TB+t, kk*128+p]; one 4MB DMA per 512-token block.
  - w  [128, NC*KT*NCH] fp8e4 : col j*KT*NCH + kk*NCH + o holds
        W_int[j*NCH+o, kk*128+p]; ternary, exact in fp8.
  - osc [128, T/128] f32      : per-token output scale rowmax_c*ws/128.

Per core: 8192 bf16(stationary x-tile) x fp8(moving 512-col weight
chunk) matmuls, fp32 PSUM accumulation over K (exact: |sum| < 2^19
<< 2^24), ACT applies the per-token scale on drain and emits bf16
(~2^-9 relative rounding on y, far inside the 2e-2 gate; host
upconverts to f32), batched 512KB y-store per 128-token tile.
"""

import sys

try:
    import concourse.bass as bass  # noqa: F401
except ImportError:
    sys.path.insert(0, "/opt/trn_rl_repo")

import numpy as np
import ml_dtypes

import concourse.tile as tile
from concourse import bacc, mybir
from concourse.bass_utils import run_bass_kernel_spmd

F32 = mybir.dt.float32
BF16 = mybir.dt.bfloat16
FP8 = mybir.dt.float8e4

EPS = 1e-5
P = 128

N_CORES = 8
B, S, K_IN, N_OUT = 4, 2048, 4096, 16384
M_FULL = B * S
N_SHARD = N_OUT // N_CORES

KT = K_IN // P  # 32 k-tiles
NCH = 512       # psum chunk (free dim per matmul)
TB = 512        # tokens per resident x slab


def build_xloop(tc, pools, wt, osc, xt_ap, y_ap, order="kk", do_xdma=True,
                do_ydma=True, x_engine="sync", y_engine="sync", y_dt=F32):
    nc = tc.nc
    xbp, yp, psy = pools
    xeng = getattr(nc, x_engine)
    yeng = getattr(nc, y_engine)
    T = xt_ap.shape[0] // P * TB
    NB = T // TB
    O = y_ap.shape[1]
    NC = O // NCH

    xb0 = None
    if not do_xdma:
        xb0 = xbp.tile([P, KT * TB], BF16, tag="xb")
        xeng.dma_start(xb0[:], xt_ap[0:P, :])

    for tb in range(NB):
        if do_xdma:
            xb = xbp.tile([P, KT * TB], BF16, tag="xb")
            xeng.dma_start(xb[:], xt_ap[tb * P : (tb + 1) * P, :])
        else:
            xb = xb0
        for m4 in range(TB // P):
            tt = tb * (TB // P) + m4
            ysb = yp.tile([P, O], y_dt, tag="y")
            if order == "kk":
                for j in range(NC):
                    ps = psy.tile([P, NCH], F32)
                    for kk in range(KT):
                        nc.tensor.matmul(
                            ps[:],
                            xb[:, kk * TB + m4 * P : kk * TB + (m4 + 1) * P],
                            wt[:, (j * KT + kk) * NCH : (j * KT + kk + 1) * NCH],
                            start=(kk == 0),
                            stop=(kk == KT - 1),
                        )
                    nc.scalar.mul(
                        ysb[:, j * NCH : (j + 1) * NCH], ps[:], osc[:, tt : tt + 1]
                    )
            else:
                pss = [psy.tile([P, NCH], F32, name=f"ps{j}", tag=f"ps{j}")
                       for j in range(NC)]
                for kk in range(KT):
                    for j in range(NC):
                        nc.tensor.matmul(
                            pss[j][:],
                            xb[:, kk * TB + m4 * P : kk * TB + (m4 + 1) * P],
                            wt[:, (j * KT + kk) * NCH : (j * KT + kk + 1) * NCH],
                            start=(kk == 0),
                            stop=(kk == KT - 1),
                        )
                for j in range(NC):
                    nc.scalar.mul(
                        ysb[:, j * NCH : (j + 1) * NCH], pss[j][:],
                        osc[:, tt : tt + 1],
                    )
            if do_ydma:
                yeng.dma_start(y_ap[tt * P : (tt + 1) * P, :], ysb[:])


def build_program(M, K, N, n_cores=N_CORES, repeat=1, debug=False, xpose="dma",
                  opts=None):
    # xpose selects the matmul issue order for A/B testing:
    #   "dma" -> chunk-outer ("kk"), "pe" -> kk-outer stationary-reuse ("jr")
    opts = dict(DEFAULT_OPTS if opts is None else opts)
    order = opts.get("order", "jr" if xpose == "pe" else "kk")
    do_xdma = opts.get("do_xdma", True)
    do_ydma = opts.get("do_ydma", True)
    x_engine = opts.get("x_engine", "sync")
    y_engine = opts.get("y_engine", "sync")
    psum_bufs = opts.get("psum_bufs", 2 if order == "jr" else 8)
    xb_bufs = opts.get("xb_bufs", 3)
    y_dt = BF16 if opts.get("y_bf16") else F32
    NB = M // TB
    NC = N // NCH
    nc = bacc.Bacc(
        "TRN2", target_bir_lowering=False, debug=debug, num_devices=n_cores
    )
    xt_ap = nc.dram_tensor("xt", [NB * P, KT * TB], BF16, kind="ExternalInput").ap()
    w_ap = nc.dram_tensor("w", [P, NC * KT * NCH], FP8, kind="ExternalInput").ap()
    osc_ap = nc.dram_tensor("osc", [P, M // P], F32, kind="ExternalInput").ap()
    y_ap = nc.dram_tensor("y", [M, N], y_dt, kind="ExternalOutput").ap()

    import contextlib

    with tile.TileContext(nc) as tc:
        ctx = contextlib.ExitStack()
        with ctx:
            const = ctx.enter_context(tc.tile_pool(name="const", bufs=1))
            wtp = ctx.enter_context(tc.tile_pool(name="wt", bufs=1))
            xbp = ctx.enter_context(tc.tile_pool(name="xb", bufs=xb_bufs))
            yp = ctx.enter_context(tc.tile_pool(name="y", bufs=3))
            psy = ctx.enter_context(
                tc.tile_pool(name="psy", bufs=psum_bufs, space="PSUM")
            )

            osc = const.tile([P, M // P], F32)
            nc.sync.dma_start(osc[:], osc_ap)
            wt = wtp.tile([P, NC * KT * NCH], FP8)
            for j in range(NC):
                nc.sync.dma_start(
                    wt[:, j * KT * NCH : (j + 1) * KT * NCH],
                    w_ap[:, j * KT * NCH : (j + 1) * KT * NCH],
                )

            pools = (xbp, yp, psy)
            kw = dict(order=order, do_xdma=do_xdma, do_ydma=do_ydma,
                      x_engine=x_engine, y_engine=y_engine, y_dt=y_dt)
            if repeat == 1:
                build_xloop(tc, pools, wt, osc, xt_ap, y_ap, **kw)
            else:
                with tc.For_i(0, repeat, 1) as _i:
                    build_xloop(tc, pools, wt, osc, xt_ap, y_ap, **kw)
    nc.compile()
    return nc


def make_inputs_np(x_full, weight_full, n_cores=N_CORES):
    """Host-side prep: quantize + pack inputs in device SBUF layout."""
    k_in = x_full.shape[-1]
    xm = np.ascontiguousarray(x_full.reshape(-1, k_in), dtype=np.float32)
    T = xm.shape[0]
    rm = np.max(np.abs(xm), axis=1, keepdims=True)
    rmc = np.maximum(rm, np.float32(EPS))
    s = (np.float32(128.0) / rmc).astype(np.float32)
    xq = np.clip(np.rint(xm * s), -128.0, 127.0)
    # pack: [tb, t, kk, p] -> [tb, p, kk, t]
    x4 = xq.reshape(T // TB, TB, KT, P)
    xpk = np.ascontiguousarray(np.transpose(x4, (0, 3, 2, 1))).reshape(
        T // TB * P, KT * TB
    ).astype(ml_dtypes.bfloat16)

    ws = np.float32(
        max(np.mean(np.abs(weight_full), dtype=np.float32), np.float32(EPS))
    )
    osc = (ws / s[:, 0]).astype(np.float32)  # [T]
    osc_mat = np.ascontiguousarray(osc.reshape(-1, P).T)  # [P, MT]

    wq = np.clip(np.rint(weight_full.astype(np.float32) / ws), -1.0, 1.0)
    nshard = weight_full.shape[0] // n_cores
    NC = nshard // NCH
    in_maps = []
    for c in range(n_cores):
        wc = wq[c * nshard : (c + 1) * nshard]  # [O, K]
        w4 = wc.reshape(NC, NCH, KT, P)  # [j, o, kk, p]
        wpk = np.ascontiguousarray(np.transpose(w4, (3, 0, 2, 1))).reshape(
            P, NC * KT * NCH
        ).astype(ml_dtypes.float8_e4m3)
        in_maps.append({"xt": xpk, "w": wpk, "osc": osc_mat})
    return in_maps, float(ws)


_NC_CACHE = {}
DEFAULT_XPOSE = "dma"
DEFAULT_OPTS = {"y_bf16": True, "xb_bufs": 4}


def _get_program():
    key = (M_FULL, K_IN, N_SHARD, N_CORES, DEFAULT_XPOSE)
    if key not in _NC_CACHE:
        _NC_CACHE[key] = build_program(
            M_FULL, K_IN, N_SHARD, N_CORES, xpose=DEFAULT_XPOSE
        )
    return _NC_CACHE[key]


def kernel(x, weight):
    x = np.asarray(x)
    weight = np.asarray(weight)
    assert x.shape == (B, S, K_IN) and weight.shape == (N_OUT, K_IN)
    nc = _get_program()
    in_maps, _ = make_inputs_np(x, weight, N_CORES)
    res = run_bass_kernel_spmd(nc, in_maps, list(range(N_CORES)))
    y = np.concatenate(
        [np.asarray(res.results[c]["y"]).astype(np.float32)
         for c in range(N_CORES)], axis=1
    )
    return np.ascontiguousarray(y.reshape(B, S, N_OUT), dtype=np.float32)
